# revision 2
# baseline (speedup 1.0000x reference)
"""Multi-head attention (B=4, S=2048, D=1024, H=16, hd=64) with RoPE on 8 trn2 cores.

Sharding: core c handles batch b=c//2, head-group hg=c%2 (8 heads, 512 features).
Each core computes y_partial.T = Wo[:, fslice] @ ctx.T for its heads; the host
sums the two partials per batch and adds bo.

v2: all matmuls N=512 (measured 259ns step), bf16 operands (f32r only for the
RoPE permute matmul), exp as [128,1024] ACT instructions, pair-major attention
pipeline with QK-projection / out-proj fillers keeping the PE continuously
busy, per-qb batched reciprocals for softmax denominators.

Device layout:
  x_sb  [128, 8, 2048]  x.T by d-chunk (p=partition within d-chunk)
  qT/kT [128, 2048] bf16 per pair (rows 0:64 head0, 64:128 head1), RoPE applied
  vt    [128, 8, 65] bf16 per k-chunk: V rows + ones column (softmax denom)
  scores.T psum [k,q] -> exp (ACT, bias -8, scale 1/8) -> es bf16
  PV: ctx.T[65, q] = vt^T @ es (row 64 = denominator)
  out:  y.T[e, q] = wo^T @ (ctxU * rden)
"""

import contextlib

import numpy as np

import concourse.bass as bass
import concourse.mybir as mybir
import concourse.tile as tile
from concourse import bacc
from concourse.bass_utils import run_bass_kernel_spmd

F32 = mybir.dt.float32
F32R = mybir.dt.float32r
BF16 = mybir.dt.bfloat16
AF = mybir.ActivationFunctionType
ADD = mybir.AluOpType.add
MULT = mybir.AluOpType.mult

B, S, D, H = 4, 2048, 1024, 16
HD = D // H            # 64
NCORES = 8
FC = D // 2            # 512 features (8 heads) per core
NH = FC // HD          # 8 heads per core (4 pairs)
NDC = D // 128         # 8 d-chunks
NFC = FC // 128        # 4 f-chunks (pairs)
NKC = S // 128         # 16 k-chunks
QB = 512               # q-block width
NQB = S // QB          # 4
EXP_BIAS = -8.0
SCALE = 1.0 / np.sqrt(HD)


def build_kernel(dump=False, repeat=1):
    nc = bacc.Bacc("TRN2", debug=False)

    xp = nc.dram_tensor("xp", [128, NDC, S], BF16, kind="ExternalInput")
    wq = nc.dram_tensor("wq", [128, NDC, FC], BF16, kind="ExternalInput")
    wk = nc.dram_tensor("wk", [128, NDC, FC], BF16, kind="ExternalInput")
    wv = nc.dram_tensor("wv", [128, NDC, FC], BF16, kind="ExternalInput")
    wo = nc.dram_tensor("wo", [128, NFC, D], BF16, kind="ExternalInput")
    bq = nc.dram_tensor("bq", [128, NFC], F32, kind="ExternalInput")
    bk = nc.dram_tensor("bk", [128, NFC], F32, kind="ExternalInput")
    bvf = nc.dram_tensor("bvf", [128, FC], F32, kind="ExternalInput")
    c2 = nc.dram_tensor("c2", [128, S], F32, kind="ExternalInput")
    s2 = nc.dram_tensor("s2", [128, S], F32, kind="ExternalInput")
    perm = nc.dram_tensor("perm", [128, 128], F32, kind="ExternalInput")
    yT = nc.dram_tensor("yT", [D, S], BF16, kind="ExternalOutput")
    if dump:
        qT_d = nc.dram_tensor("qT_d", [FC, S], BF16, kind="ExternalOutput")
        kT_d = nc.dram_tensor("kT_d", [FC, S], BF16, kind="ExternalOutput")
        vt_d = nc.dram_tensor("vt_d", [S, NH, HD + 1], BF16,
                              kind="ExternalOutput")
        cu_d = nc.dram_tensor("cu_d", [FC, S], BF16, kind="ExternalOutput")

    with tile.TileContext(nc) as tc:
      for _rep in range(repeat):
       with contextlib.ExitStack() as ctx:
        const = ctx.enter_context(tc.tile_pool(name="const", bufs=1))
        big = ctx.enter_context(tc.tile_pool(name="big", bufs=1))

        perm_sb = const.tile([128, 128], F32R, name="perm_sb")
        c2_sb = const.tile([128, S], F32, name="c2_sb")
        s2_sb = const.tile([128, S], F32, name="s2_sb")
        bqs = const.tile([128, NFC], F32, name="bqs")
        bks = const.tile([128, NFC], F32, name="bks")
        bvs = const.tile([128, FC], F32, name="bvs")
        wo_sb = const.tile([128, NFC, D], BF16, name="wo_sb")
        ebias = const.tile([128, 1], F32, name="ebias")
        nc.vector.memset(ebias, EXP_BIAS)

        qT = [big.tile([128, S], BF16, name=f"qT{i}") for i in range(NFC)]
        kT = [big.tile([128, S], BF16, name=f"kT{i}") for i in range(NFC)]
        vt = [big.tile([128, NH, HD + 1], BF16, name=f"vt{k}")
              for k in range(NKC)]
        # ctxU/den are split per q-block: the tile framework's dependency
        # tracking is per-tile, so a single [128, S] tile would serialize
        # this q-block's normalize against later drains of other q-blocks.
        ctxU = [[big.tile([128, QB], BF16, name=f"ctxU{i}_{j}")
                 for j in range(NQB)] for i in range(NFC)]
        # engine APs may only start at partitions {0,32,64,96}: spread the 8
        # denominator rows over two tiles at those bases; unused rows stay 1.0
        # so the full-tile reciprocal remains finite.
        den = [[big.tile([128, QB], BF16, name=f"den{i}_{j}")
                for j in range(NQB)] for i in range(2)]
        for i in range(2):
            for j in range(NQB):
                nc.vector.memset(den[i][j], 1.0)
        for kc in range(NKC):
            nc.vector.memset(vt[kc][:, :, HD:HD + 1], 1.0)

        yT_r = yT[:].rearrange("(c p) s -> c p s", p=128)

        esp = ctx.enter_context(tc.tile_pool(name="esp", bufs=1))
        bps = ctx.enter_context(tc.tile_pool(name="bps", bufs=1, space="PSUM"))
        actx = ctx.enter_context(contextlib.ExitStack())
        sbA = actx.enter_context(tc.tile_pool(name="sbA", bufs=1))
        qkps = actx.enter_context(tc.tile_pool(name="qkps", bufs=1,
                                               space="PSUM"))

        x_sb = sbA.tile([128, NDC, S], BF16, name="x_sb")
        wq_sb = sbA.tile([128, NDC, FC], BF16, name="wq_sb")
        wk_sb = sbA.tile([128, NDC, FC], BF16, name="wk_sb")
        wv_sb = sbA.tile([128, NDC, FC], BF16, name="wv_sb")

        # each dma_start costs ~0.65us of serial issue time on its queue, so
        # keep the count low and priority-ordered on sync; small/late tensors
        # issue from the otherwise-idle gpsimd queue. x lands in s-major
        # slabs (all d-chunks of an s range) so the V projection can start
        # on slab 0 while the rest stream in.
        XSLAB = S // NDC
        nc.sync.dma_start(out=wv_sb[:, 0:NDC // 2, :],
                          in_=wv[:][:, 0:NDC // 2, :])
        nc.sync.dma_start(out=wv_sb[:, NDC // 2:NDC, :],
                          in_=wv[:][:, NDC // 2:NDC, :])
        for i in range(NDC):
            ssl = slice(i * XSLAB, (i + 1) * XSLAB)
            nc.sync.dma_start(out=x_sb[:, :, ssl], in_=xp[:][:, :, ssl])
        nc.sync.dma_start(out=wq_sb, in_=wq[:])
        nc.sync.dma_start(out=wk_sb, in_=wk[:])
        nc.gpsimd.dma_start(out=bqs, in_=bq[:])
        nc.gpsimd.dma_start(out=bks, in_=bk[:])
        nc.gpsimd.dma_start(out=bvs, in_=bvf[:])
        nc.gpsimd.dma_start(out=c2_sb, in_=c2[:])
        nc.gpsimd.dma_start(out=s2_sb, in_=s2[:])
        nc.gpsimd.dma_start(out=perm_sb, in_=perm[:].bitcast(F32R))
        nc.gpsimd.dma_start(out=wo_sb, in_=wo[:])

        def qk_subunit(t_idx, fc, sb):
            """One (q|k, fc, sb) projection+RoPE piece: 9 matmuls + 4 DVE."""
            w_t = wq_sb if t_idx == 0 else wk_sb
            bias_t = bqs if t_idx == 0 else bks
            out_t = (qT if t_idx == 0 else kT)[fc]
            ssl = slice(sb * QB, (sb + 1) * QB)
            pp = qkps.tile([128, QB], F32, name="pp", tag="pp", bufs=2)
            for d in range(NDC):
                nc.tensor.matmul(
                    pp, w_t[:, d, fc * 128:(fc + 1) * 128],
                    x_sb[:, d, ssl], start=(d == 0), stop=(d == NDC - 1))
            praw = sbA.tile([128, QB], F32R, name="praw", tag="praw", bufs=2)
            nc.vector.tensor_scalar(
                praw, pp, bias_t[:, fc:fc + 1], None, op0=ADD)
            # sw reuses the pp tag's banks: pp's only reader (praw) runs
            # before sw needs the older buffer, so no extra dependency.
            sw = qkps.tile([128, QB], F32, name="sw", tag="pp", bufs=2)
            nc.tensor.matmul(sw, perm_sb, praw, start=True, stop=True)
            prod = sbA.tile([128, QB], BF16, name="prod", tag="prod", bufs=1)
            nc.vector.tensor_tensor(prod, sw, s2_sb[:, ssl], op=MULT)
            tq = sbA.tile([128, QB], BF16, name="tq", tag="tq", bufs=1)
            nc.vector.tensor_tensor(tq, praw, c2_sb[:, ssl], op=MULT)
            nc.vector.tensor_tensor(out_t[:, ssl], tq, prod, op=ADD)

        def qk_pair(p):
            return [(t, p, sb) for t in range(2) for sb in range(NQB)]

        # ---------------- attention ----------------
        out_emitted = [False] * NQB
        cpool = {}

        def out_proj_chunks(qb, tail=False):
            """Out-projection for one q-block as 8 filler pieces (cost ~1us).

            In the tail (no other PE work) the 2-buffer op rotation exposes
            the matmul->CAST->psum-free latency (~3.4us/chunk); alternating
            with the idle sc tag doubles the buffers in rotation.
            """
            qsl = slice(qb * QB, (qb + 1) * QB)

            def mk(ec):
                def f():
                    if tail and ec % 2 == 1:
                        sc_t = bps.tile([128, 2, QB], F32, name="sc",
                                        tag="sc", bufs=2)
                        op = sc_t[:, 0, :]
                    else:
                        op = cpool["cps"].tile([128, QB], F32, name="op",
                                               tag="op", bufs=2)
                    for fc in range(NFC):
                        nc.tensor.matmul(
                            op, wo_sb[:, fc, ec * 128:(ec + 1) * 128],
                            ctxU[fc][qb],
                            start=(fc == 0), stop=(fc == NFC - 1))
                    ysb = big.tile([128, QB], BF16, name="ysb", tag="ysb",
                                   bufs=2)
                    if tail and ec % 2 == 1:
                        # ACT is idle once the exp stream ends; Copy shares
                        # the exp table so there is no table reload
                        nc.scalar.activation(ysb, op, AF.Copy, scale=1.0)
                    else:
                        nc.vector.tensor_copy(ysb, op)
                    eng = nc.sync if ec % 2 == 0 else nc.gpsimd
                    eng.dma_start(out=yT_r[ec, :, qsl], in_=ysb)
                return f

            out_emitted[qb] = True
            return [(1.1, mk(ec)) for ec in range(NDC)]

        def sc_group_fns(p, h, qb, es):
            """8 callables, each: 2 score matmuls + 1 exp into es."""
            qsl = slice(qb * QB, (qb + 1) * QB)

            def mk(g):
                def f():
                    sc_t = bps.tile([128, 2, QB], F32, name="sc", tag="sc",
                                    bufs=2)
                    for j in range(2):
                        kc = 2 * g + j
                        nc.tensor.matmul(
                            sc_t[:, j, :],
                            kT[p][h * 64:(h + 1) * 64,
                                  kc * 128:(kc + 1) * 128],
                            qT[p][h * 64:(h + 1) * 64, qsl],
                            start=True, stop=True, tile_position=(h * 64, 0))
                    nc.scalar.activation(
                        es[:, 2 * g:2 * g + 2, :], sc_t, AF.Exp,
                        bias=ebias, scale=SCALE)
                return f

            return [mk(g) for g in range(NKC // 2)]

        def pv_chunk_fns(p, h, qb, es):
            """4 PV chunks (4 kc each) + drain, for interleaved emission."""
            qsl = slice(qb * QB, (qb + 1) * QB)
            hh = p * 2 + h
            ctx_ref = [None]

            def mk(c):
                def f():
                    if c == 0:
                        ctx_ref[0] = bps.tile([128, QB], F32, name="ctx",
                                              tag="ctx", bufs=2)
                    for kc in range(4 * c, 4 * c + 4):
                        nc.tensor.matmul(
                            ctx_ref[0][0:HD + 1, :], vt[kc][:, hh, :],
                            es[:, kc, :],
                            start=(kc == 0), stop=(kc == NKC - 1))
                return f

            def drain():
                nc.vector.tensor_copy(
                    ctxU[p][qb][h * 64:(h + 1) * 64, :], ctx_ref[0][0:HD, :])
                db = (hh % 4) * 32
                nc.vector.tensor_copy(
                    den[hh // 4][qb][db:db + 1, :], ctx_ref[0][HD:HD + 1, :])

            return [mk(c) for c in range(4)], drain

        def norm_pieces(qb):
            """Normalize for one q-block as 4 low-PE-cost pieces: two
            reciprocals (spread so the DVE queue never bursts) and two
            scale groups of 4 heads each."""
            rdens = {}

            def recip(half):
                def f():
                    r = big.tile([128, QB], BF16, name="rden", tag="rden",
                                 bufs=2)
                    with nc.allow_low_precision(reason="softmax denom recip"):
                        nc.vector.reciprocal(r, den[half][qb])
                    # one strided DMA hops rows {0,32,64,96} down to a
                    # base-0 temp (partition_broadcast only reads part. 0)
                    d4 = big.tile([1, 4, QB], BF16, name="denr", tag="denr",
                                  bufs=2)
                    nc.sync.dma_start(out=d4, in_=r[0:128:32, :])
                    rdens[half] = d4
                return f

            def scale4(half):
                def f():
                    d4 = rdens[half]
                    for j in range(4):
                        hh = half * 4 + j
                        p, h = hh // 2, hh % 2
                        # full-height broadcast so the in-place scale reads
                        # both operands at the same base partition
                        denb = big.tile([128, QB], BF16, name="denb",
                                        tag="denb", bufs=2)
                        nc.gpsimd.partition_broadcast(denb, d4[0:1, j, :])
                        nc.vector.tensor_tensor(
                            ctxU[p][qb][h * 64:(h + 1) * 64, :],
                            ctxU[p][qb][h * 64:(h + 1) * 64, :],
                            denb[h * 64:(h + 1) * 64, :], op=MULT)
                return f

            return [recip(0), recip(1), scale4(0), scale4(1)]

        # Interleaved slot scheduler: per slot emit the next unit's 8
        # sc-groups with the previous unit's 4 PV chunks woven between the
        # early groups and filler pieces (QK projections of the next pair,
        # or out-proj chunks in the last window) between the late groups.
        # preamble: V starts as soon as its x slabs land; pair-0 projection
        # subunits weave in once wq/wk arrive, and the first unit's scores
        # follow so the ACT engine starts its exp stream early.
        def v_subunit(sc):
            psv = qkps.tile([128, FC], F32, name="psv", tag="pp", bufs=2)
            for d in range(NDC):
                nc.tensor.matmul(
                    psv, x_sb[:, d, sc * 128:(sc + 1) * 128],
                    wv_sb[:, d, :], start=(d == 0), stop=(d == NDC - 1))
            nc.vector.tensor_tensor(
                vt[sc][:, :, 0:HD],
                psv.rearrange("p (h e) -> p h e", e=HD),
                bvs.rearrange("p (h e) -> p h e", e=HD), op=ADD)

        for sc in range(NKC):
            v_subunit(sc)
        for args in qk_pair(0):
            qk_subunit(*args)
        es0 = esp.tile([128, NKC, QB], BF16, name="es", tag="es", bufs=2)
        for f in sc_group_fns(0, 0, 0, es0):
            f()

        filler_q = []   # PE filler pieces: (cost_us, fn)
        aux_q = []      # low-PE-cost pieces (normalize), 1 per slot
        pv_prev, drain_prev = pv_chunk_fns(0, 0, 0, es0)
        for p in range(NFC):
            last = p == NFC - 1
            if not last:
                filler_q.extend(
                    (2.3, (lambda a: lambda: qk_subunit(*a))(a))
                    for a in qk_pair(p + 1))
            units = [(h, qb) for qb in range(NQB) for h in range(2)]
            if p == 0:
                units = units[1:]
            for h, qb in units:
                es = esp.tile([128, NKC, QB], BF16, name="es", tag="es",
                              bufs=2)
                scs = sc_group_fns(p, h, qb, es)
                budget = 4.8
                for g in range(8):
                    scs[g]()
                    if g < 4:
                        if pv_prev is not None:
                            pv_prev[g]()
                        if filler_q and budget > 0:
                            cost, fn = filler_q.pop(0)
                            fn()
                            budget -= cost
                if drain_prev is not None:
                    drain_prev()
                if aux_q:
                    aux_q.pop(0)()
                pv_prev, drain_prev = pv_chunk_fns(p, h, qb, es)
                if last and h == 1:
                    def mk_push(qb=qb):
                        def f():
                            filler_q.extend(
                                out_proj_chunks(qb, tail=(qb == NQB - 1)))
                        return f
                    aux_q.extend(norm_pieces(qb))
                    aux_q.append(mk_push())
            if p == NFC - 2:
                # QK fillers must finish inside their window; x/weights + qk
                # psum then free up banks for the out-proj psum pool
                for cost, fn in filler_q:
                    fn()
                filler_q = []
                actx.close()
                cpool["cps"] = ctx.enter_context(
                    tc.tile_pool(name="cps", bufs=1, space="PSUM"))

        # tail: last unit's PV + drain, then remaining normalize + out-proj
        for g in range(4):
            pv_prev[g]()
        for cost, fn in filler_q:
            fn()
        filler_q = []
        drain_prev()
        for fn in aux_q:
            fn()
        aux_q = []
        for cost, fn in filler_q:
            fn()
        for qb in range(NQB):
            if not out_emitted[qb]:
                for cost, fn in out_proj_chunks(qb, tail=True):
                    fn()

        if dump:
            qd_r = qT_d[:].rearrange("(c p) s -> c p s", p=128)
            kd_r = kT_d[:].rearrange("(c p) s -> c p s", p=128)
            cd_r = cu_d[:].rearrange("(c p) s -> c p s", p=128)
            vd_r = vt_d[:].rearrange("(c p) h e -> c p h e", p=128)
            for fc in range(NFC):
                nc.sync.dma_start(out=qd_r[fc], in_=qT[fc])
                nc.sync.dma_start(out=kd_r[fc], in_=kT[fc])
                for j in range(NQB):
                    nc.sync.dma_start(
                        out=cd_r[fc][:, j * QB:(j + 1) * QB], in_=ctxU[fc][j])
            for kc in range(NKC):
                nc.sync.dma_start(out=vd_r[kc], in_=vt[kc])

    nc.finalize()
    return nc


def _rope_tables():
    inv_freq = 1.0 / (10000.0 ** (np.arange(0, HD, 2, dtype=np.float64) / HD))
    pos = np.arange(S, dtype=np.float64)
    sinu = pos[None, :] * inv_freq[:, None]          # [32, S]
    c = np.sin(sinu).astype(np.float32)              # torch code calls this 'cos'
    s = np.cos(sinu).astype(np.float32)              # and this 'sin'
    c2 = np.tile(c, (4, 1))                          # [128, S]
    s2 = np.concatenate([-s, s, -s, s], axis=0)      # [128, S]
    return np.ascontiguousarray(c2), np.ascontiguousarray(s2)


def make_in_maps(inp):
    """inp: dict of full numpy inputs -> list of 8 per-core input maps."""
    import ml_dtypes
    BF = ml_dtypes.bfloat16
    c2, s2 = _rope_tables()
    pm = np.zeros((128, 128), np.float32)
    for h in range(2):
        for j in range(32):
            pm[h * 64 + 32 + j, h * 64 + j] = 1.0      # out j <- in k
            pm[h * 64 + j, h * 64 + 32 + j] = 1.0
    maps = []
    for c in range(NCORES):
        b, hg = c // 2, c % 2
        fsl = slice(hg * FC, (hg + 1) * FC)
        x = np.asarray(inp["hidden_states"][b], np.float32)
        xp_ = np.ascontiguousarray(
            x.T.reshape(NDC, 128, S).transpose(1, 0, 2)).astype(BF)
        wqp = np.ascontiguousarray(
            np.asarray(inp["Wq"], np.float32)[fsl].T.reshape(NDC, 128, FC)
            .transpose(1, 0, 2)).astype(BF)
        wkp = np.ascontiguousarray(
            np.asarray(inp["Wk"], np.float32)[fsl].T.reshape(NDC, 128, FC)
            .transpose(1, 0, 2)).astype(BF)
        wvp = np.ascontiguousarray(
            np.asarray(inp["Wv"], np.float32)[fsl].T.reshape(NDC, 128, FC)
            .transpose(1, 0, 2)).astype(BF)
        wop = np.ascontiguousarray(
            np.asarray(inp["Wo"], np.float32)[:, fsl].T.reshape(NFC, 128, D)
            .transpose(1, 0, 2)).astype(BF)
        bqp = np.ascontiguousarray(
            np.asarray(inp["bq"], np.float32)[fsl].reshape(NFC, 128).T)
        bkp = np.ascontiguousarray(
            np.asarray(inp["bk"], np.float32)[fsl].reshape(NFC, 128).T)
        bvp = np.ascontiguousarray(np.broadcast_to(
            np.asarray(inp["bv"], np.float32)[fsl][None, :], (128, FC)))
        maps.append({
            "xp": xp_, "wq": wqp, "wk": wkp, "wv": wvp, "wo": wop,
            "bq": bqp, "bk": bkp, "bvf": bvp,
            "c2": c2, "s2": s2, "perm": pm,
        })
    return maps


_NC_CACHE = {}


def kernel(hidden_states, Wq, bq, Wk, bk, Wv, bv, Wo, bo):
    if "nc" not in _NC_CACHE:
        _NC_CACHE["nc"] = build_kernel()
    nc = _NC_CACHE["nc"]
    in_maps = make_in_maps({
        "hidden_states": hidden_states, "Wq": Wq, "bq": bq, "Wk": Wk, "bk": bk,
        "Wv": Wv, "bv": bv, "Wo": Wo,
    })
    res = run_bass_kernel_spmd(nc, in_maps, list(range(NCORES)))
    bo = np.asarray(bo, np.float32)
    out = np.empty((B, S, D), np.float32)
    for b in range(B):
        acc = (np.asarray(res.results[2 * b]["yT"]).astype(np.float32)
               + np.asarray(res.results[2 * b + 1]["yT"]).astype(np.float32))
        out[b] = acc.T + bo[None, :]
    return out
